# revision 19
# baseline (speedup 1.0000x reference)
"""NeighborMLPConvLayer Trainium2 kernel (v3).

Strategy (8 NeuronCores, SPMD, edge-parallel):
  - Edges (sorted by destination segment) are split into 8 equal contiguous
    ranges; boundary segments are fixed up by a host-side overlap-add.
  - Per core, edges are packed into 128-edge "chunks"; a chunk never spans
    more than 16 distinct segments (cut + pad otherwise, which is rare at
    avg degree 32).  12 chunks form a 1536-slot window.
  - The HOST gathers features per edge into a dense bf16 stream
    comb[65, slots]: rows 0-31 in_features[idx], rows 32-63
    out_features[seg], row 64 ones (injects b1 via W1cat row 64).  Dense
    streams run at full DMA bandwidth (no per-row gather descriptors).
  - Device per chunk: h = comb_chunk.T @ W1cat (K=65) -> PSUM [128e, H];
    one gelu per window on Act (the bottleneck engine) -> SBUF bf16.
  - Segment-sum BEFORE W2 (linearity): hsT[H, 16] = h_chunk.T @ onehot16
    where onehot16 is a per-chunk [128e, 16s] fp8 one-hot (16 B/edge).
    Per window the 12 chunk slabs land in one PSUM scratch tile.
  - M2: y[96, O] = hsT_sb.T @ W2 twice per window (192 chunk-seg rows).
  - The loop is software-pipelined with a 1-window skew (PE is in-order:
    segsum(k-1), which waits on gelu(k-1), is emitted after M1(k) so the
    gelu hides behind the next window's M1).
  - Host: overlap-add chunk slabs (base + 16 rows each) into out[M, O]
    via per-column bincount, then divide by counts and add b2.
"""

import sys

sys.path.insert(0, "/opt/trn_rl_repo")

import numpy as np
import ml_dtypes

BF16 = ml_dtypes.bfloat16
FP8 = ml_dtypes.float8_e4m3

# Problem geometry (hardcoded per the task contract).
N = 50000
M = 50000
C = 32
H = 128
O = 64
E = 1_600_000
NCORES = 8

CHUNK = 128            # edges per chunk (PE partition dim)
SEGW = 16              # max segments spanned by one chunk (one-hot width)
CPW = 12               # chunks per window (PSUM: 12*128*4B = 3 banks)
WIN = CHUNK * CPW      # 1536 edge slots per window
GRP = 2                # windows per DMA group
KC = C + C + 1         # comb rows: rep(32) + slf(32) + ones(1)
M2R = CPW * SEGW // 2  # 96 output rows per M2 matmul

_prog_cache = {}


# ----------------------------------------------------------------- host prep

def _cut_chunks(seg_c):
    """Greedy 128-edge chunks, each spanning < SEGW segments.

    Returns (starts, ends, bases) arrays.
    """
    n = seg_c.shape[0]
    starts, ends, bases = [], [], []
    p = 0
    while p < n:
        b = int(seg_c[p])
        q = int(np.searchsorted(seg_c, b + SEGW, side="left"))
        cut = min(p + CHUNK, q, n)
        assert cut > p
        starts.append(p)
        ends.append(cut)
        bases.append(b)
        p = cut
    return (np.asarray(starts, np.int64), np.asarray(ends, np.int64),
            np.asarray(bases, np.int64))


def _host_prep(in_features, out_features, W1, b1, W2, b2,
               neighbors_index, neighbors_row_splits):
    rs = np.asarray(neighbors_row_splits).astype(np.int64)
    idx_all = np.asarray(neighbors_index).astype(np.int64)
    counts = np.diff(rs)
    seg_all = np.repeat(np.arange(M, dtype=np.int64), counts)

    bounds = [round(k * E / NCORES) for k in range(NCORES + 1)]

    cores = []
    nwin = 0
    for k in range(NCORES):
        lo, hi = bounds[k], bounds[k + 1]
        seg_c = seg_all[lo:hi]
        starts, ends, bases = _cut_chunks(seg_c)
        nch = starts.shape[0]
        nwin = max(nwin, -(-nch // CPW))
        cores.append((idx_all[lo:hi], seg_c, starts, ends, bases))

    inF = np.asarray(in_features, np.float32).astype(BF16)
    outF = np.asarray(out_features, np.float32).astype(BF16)

    w1 = np.asarray(W1, np.float32)
    w1cat = np.concatenate([w1, np.asarray(b1, np.float32).reshape(1, H)], 0)
    consts = dict(
        w1cat=np.ascontiguousarray(w1cat).astype(BF16),
        w2=np.asarray(W2, np.float32).astype(BF16),
    )

    in_maps = []
    metas = []
    for k in range(NCORES):
        idx_c, seg_c, starts, ends, bases = cores[k]
        n = idx_c.shape[0]
        nch = starts.shape[0]
        ncs = ends - starts
        chunk_ids = np.repeat(np.arange(nch, dtype=np.int64), ncs)
        within = np.arange(n, dtype=np.int64) - np.repeat(starts, ncs)
        slots = chunk_ids * CHUNK + within

        comb = np.zeros((KC, nwin * WIN), BF16)
        comb[0:C, slots] = inF[idx_c].T
        comb[C:2 * C, slots] = outF[seg_c].T
        comb[2 * C, slots] = np.float32(1.0)

        sloc = seg_c - bases[chunk_ids]
        assert int(sloc.max()) < SEGW
        sme = np.zeros((CHUNK, nwin * CPW * SEGW), FP8)
        sme[within, chunk_ids * SEGW + sloc] = np.float32(1.0)

        bases_full = np.zeros(nwin * CPW, np.int64)
        bases_full[:nch] = bases

        in_maps.append(dict(comb=comb, sme=sme, **consts))
        metas.append(dict(bases=bases_full, nch=nch))

    return in_maps, metas, nwin, counts


# ------------------------------------------------------------ device program

def _build_program(nwin):
    import concourse.bacc as bacc
    import concourse.mybir as mybir
    import concourse.tile as tile

    dt = mybir.dt
    nc = bacc.Bacc("TRN2", target_bir_lowering=False, debug=False)

    d_comb = nc.dram_tensor("comb", [KC, nwin * WIN], dt.bfloat16,
                            kind="ExternalInput")
    d_sme = nc.dram_tensor("sme", [CHUNK, nwin * CPW * SEGW], dt.float8e4,
                           kind="ExternalInput")
    d_w1cat = nc.dram_tensor("w1cat", [KC, H], dt.bfloat16,
                             kind="ExternalInput")
    d_w2 = nc.dram_tensor("w2", [H, O], dt.bfloat16, kind="ExternalInput")
    d_y = nc.dram_tensor("yout", [M2R, nwin * 2 * O], dt.float32,
                         kind="ExternalOutput")

    from contextlib import ExitStack

    with tile.TileContext(nc) as tc, ExitStack() as ctx:
        cpool = ctx.enter_context(tc.tile_pool(name="consts", bufs=1))
        gpool = ctx.enter_context(tc.tile_pool(name="stream", bufs=3))
        hpool = ctx.enter_context(tc.tile_pool(name="hsb", bufs=3))
        spool = ctx.enter_context(tc.tile_pool(name="small", bufs=3))
        ypool = ctx.enter_context(tc.tile_pool(name="ystage", bufs=3))
        hpsum = ctx.enter_context(tc.tile_pool(name="hpsum", bufs=2,
                                               space="PSUM"))
        wpsum = ctx.enter_context(tc.tile_pool(name="wpsum", bufs=2,
                                               space="PSUM"))

        w1_sb = cpool.tile([KC, H], dt.bfloat16, tag="w1")
        w2_sb = cpool.tile([H, O], dt.bfloat16, tag="w2")
        # consts issue on Act's (idle) sequencer so the first comb stream is
        # not queued behind them on SP.
        nc.scalar.dma_start(out=w1_sb[:], in_=d_w1cat[:])
        nc.scalar.dma_start(out=w2_sb[:], in_=d_w2[:])

        # Warm the Gelu activation table while the first streams are in
        # flight (table load is ~1.3us and otherwise serializes before the
        # first real gelu).
        warm = cpool.tile([1, 2], dt.bfloat16, tag="warm")
        nc.gpsimd.memset(warm[:], 0.0)
        nc.scalar.activation(warm[:], warm[:],
                             func=mybir.ActivationFunctionType.Gelu,
                             bias=0.0, scale=1.0)

        # group g covers windows [g*GRP, g*GRP+gsz); the last group may be
        # smaller than GRP when nwin is not a multiple of GRP.
        ngrp = -(-nwin // GRP)
        gsize = [min(GRP, nwin - g * GRP) for g in range(ngrp)]

        tiles = {}   # group -> (comb_t, sme_t)
        ysbs = {}    # group -> y_sb staging tile
        pend = None  # (k, h_ps, h_sb) waiting for its segsum/M2 phase

        def fetch_group(g):
            gsz = gsize[g]
            w0 = g * GRP
            comb_t = gpool.tile([KC, gsz * WIN], dt.bfloat16,
                                tag=f"comb{gsz}", name=f"comb{g}")
            if g == 0:
                # split per window so the first M1 starts after 1/GRP of the
                # stream has landed (range-tracked dependencies).
                for w in range(gsz):
                    nc.sync.dma_start(
                        out=comb_t[:, w * WIN:(w + 1) * WIN],
                        in_=d_comb[:, (w0 + w) * WIN:(w0 + w + 1) * WIN])
            else:
                nc.sync.dma_start(
                    out=comb_t[:],
                    in_=d_comb[:, w0 * WIN:(w0 + gsz) * WIN])
            sme_t = gpool.tile([CHUNK, gsz * CPW * SEGW], dt.float8e4,
                               tag=f"sme{gsz}", name=f"sme{g}")
            nc.gpsimd.dma_start(
                out=sme_t[:],
                in_=d_sme[:, w0 * CPW * SEGW:(w0 + gsz) * CPW * SEGW])
            tiles[g] = (comb_t, sme_t)

        def finish_window(k, h_ps, h_sb):
            g = k // GRP
            w = k % GRP
            gsz = gsize[g]
            sme_t = tiles[g][1]
            # combined scratch: hsT [128, CPW*16] f32 + y [96, 2, 64] f32 in
            # one PSUM bank.
            scr = wpsum.tile([CHUNK, CPW * SEGW + 2 * O], dt.float32,
                             tag="scr", name=f"scr{k}")
            hsT_ps = scr[:, 0:CPW * SEGW]
            for c in range(CPW):
                nc.tensor.matmul(
                    hsT_ps[:, c * SEGW:(c + 1) * SEGW],
                    lhsT=h_sb[:, c, :],
                    rhs=sme_t[:, (w * CPW + c) * SEGW:
                              (w * CPW + c + 1) * SEGW],
                    start=True, stop=True,
                    skip_group_check=True,
                )
            hsT_sb = spool.tile([H, CPW, SEGW], dt.bfloat16, tag="hsTsb")
            nc.vector.tensor_copy(out=hsT_sb[:],
                                  in_=hsT_ps.rearrange(
                                      "p (a b) -> p a b", a=CPW))
            y_ps = scr[0:M2R, CPW * SEGW:].rearrange("p (a b) -> p a b", a=2)
            for s2 in range(2):
                nc.tensor.matmul(
                    y_ps[:, s2, :],
                    lhsT=hsT_sb[:, s2 * (CPW // 2):(s2 + 1) * (CPW // 2), :],
                    rhs=w2_sb[:],
                    start=True, stop=True,
                    skip_group_check=True,
                )
            if g not in ysbs:
                ysbs[g] = ypool.tile([M2R, gsz * 2, O], dt.float32,
                                     tag=f"ysb{gsz}", name=f"ysb{g}")
            nc.vector.tensor_copy(out=ysbs[g][:, w * 2:(w + 1) * 2, :],
                                  in_=y_ps)
            if w == gsz - 1:
                y0 = g * GRP * 2 * O
                nc.sync.dma_start(
                    out=d_y[:, y0:y0 + gsz * 2 * O],
                    in_=ysbs.pop(g)[:])
                tiles.pop(g)

        for k in range(nwin):
            g, w = k // GRP, k % GRP
            if w == 0:
                fetch_group(g)
            comb_t = tiles[g][0]
            h_ps = hpsum.tile([CHUNK, CPW, H], dt.float32, tag="h")
            for c in range(CPW):
                e0 = w * WIN + c * CHUNK
                nc.tensor.matmul(
                    h_ps[:, c, :],
                    lhsT=comb_t[:, e0:e0 + CHUNK],
                    rhs=w1_sb[:],
                    start=True, stop=True,
                )
            h_sb = hpool.tile([CHUNK, CPW, H], dt.bfloat16, tag="hsb")
            nc.scalar.activation(
                h_sb[:], h_ps[:],
                func=mybir.ActivationFunctionType.Gelu,
                bias=0.0, scale=1.0,
            )
            if pend is not None:
                finish_window(*pend)
            pend = (k, h_ps, h_sb)
        finish_window(*pend)

    nc.compile()
    return nc


# ------------------------------------------------------------------- runner

LAST_RESULT = None


def kernel(in_features, out_features, W1, b1, W2, b2,
           neighbors_index, neighbors_row_splits):
    import os
    from concourse.bass_utils import run_bass_kernel_spmd

    in_maps, metas, nwin, counts = _host_prep(
        in_features, out_features, W1, b1, W2, b2,
        neighbors_index, neighbors_row_splits,
    )

    if nwin not in _prog_cache:
        _prog_cache[nwin] = _build_program(nwin)
    nc = _prog_cache[nwin]

    trace = bool(os.environ.get("KERNEL_TRACE"))
    if trace:
        try:
            import antenv.axon_hooks  # noqa: F401
        except ImportError:
            trace = False
    res = run_bass_kernel_spmd(nc, in_maps, core_ids=list(range(NCORES)),
                               trace=trace)
    global LAST_RESULT
    LAST_RESULT = res
    outs = res.results

    acc = np.zeros((M, O), np.float64)
    for k in range(NCORES):
        # yout [96, nwin*2, O] -> rows r = (w, s2, p) map to
        # chunk = w*CPW + s2*6 + p//16, seg = bases[chunk] + p % 16.
        y = np.asarray(outs[k]["yout"], np.float32)
        y = y.reshape(M2R, nwin, 2, O).transpose(1, 2, 0, 3)
        y = np.ascontiguousarray(y).reshape(nwin * 2 * M2R, O)
        p = np.tile(np.arange(M2R), nwin * 2)
        s2 = np.tile(np.repeat(np.arange(2), M2R), nwin)
        w = np.repeat(np.arange(nwin), 2 * M2R)
        chunk = w * CPW + s2 * (CPW // 2) + p // SEGW
        gidx = metas[k]["bases"][chunk] + p % SEGW
        for o in range(O):
            # rows past a chunk's actual span are all-zero; indices may run
            # past M-1 for chunks near the end — truncate.
            acc[:, o] += np.bincount(gidx, weights=y[:, o].astype(np.float64),
                                     minlength=M)[:M]

    denom = np.maximum(counts, 1).astype(np.float64)
    out = (acc / denom[:, None]).astype(np.float32)
    b2v = np.asarray(b2, np.float32)
    out += b2v[None, :] * (counts > 0)[:, None].astype(np.float32)
    return out


# revision 30
# speedup vs baseline: 1.0038x; 1.0038x over previous
"""NeighborMLPConvLayer Trainium2 kernel (v3).

Strategy (8 NeuronCores, SPMD, edge-parallel):
  - Edges (sorted by destination segment) are split into 8 equal contiguous
    ranges; boundary segments are fixed up by a host-side overlap-add.
  - Per core, edges are packed into 128-edge "chunks"; a chunk never spans
    more than 16 distinct segments (cut + pad otherwise, which is rare at
    avg degree 32).  12 chunks form a 1536-slot window.
  - The HOST gathers features per edge into a dense bf16 stream
    comb[65, slots]: rows 0-31 in_features[idx], rows 32-63
    out_features[seg], row 64 ones (injects b1 via W1cat row 64).  Dense
    streams run at full DMA bandwidth (no per-row gather descriptors).
  - Device per chunk: h = comb_chunk.T @ W1cat (K=65) -> PSUM [128e, H];
    one gelu per window on Act (the bottleneck engine) -> SBUF bf16.
  - Segment-sum BEFORE W2 (linearity): hsT[H, 16] = h_chunk.T @ onehot16
    where onehot16 is a per-chunk [128e, 16s] fp8 one-hot (16 B/edge).
    Per window the 12 chunk slabs land in one PSUM scratch tile.
  - M2: y[96, O] = hsT_sb.T @ W2 twice per window (192 chunk-seg rows).
  - The loop is software-pipelined with a 1-window skew (PE is in-order:
    segsum(k-1), which waits on gelu(k-1), is emitted after M1(k) so the
    gelu hides behind the next window's M1).
  - Host: overlap-add chunk slabs (base + 16 rows each) into out[M, O]
    via per-column bincount, then divide by counts and add b2.
"""

import sys

sys.path.insert(0, "/opt/trn_rl_repo")

import numpy as np
import ml_dtypes

BF16 = ml_dtypes.bfloat16
FP8 = ml_dtypes.float8_e4m3

# Problem geometry (hardcoded per the task contract).
N = 50000
M = 50000
C = 32
H = 128
O = 64
E = 1_600_000
NCORES = 8

CHUNK = 128            # edges per chunk (PE partition dim)
SEGW = 16              # max segments spanned by one chunk (one-hot width)
CPW = 12               # chunks per window (PSUM: 12*128*4B = 3 banks)
WIN = CHUNK * CPW      # 1536 edge slots per window
GRP = 2                # windows per DMA group
KC = C + C + 1         # comb rows: rep(32) + slf(32) + ones(1)
M2R = CPW * SEGW // 2  # 96 output rows per M2 matmul

_prog_cache = {}


# ----------------------------------------------------------------- host prep

def _cut_chunks(seg_c):
    """Greedy 128-edge chunks, each spanning < SEGW segments.

    Returns (starts, ends, bases) arrays.
    """
    n = seg_c.shape[0]
    starts, ends, bases = [], [], []
    p = 0
    while p < n:
        b = int(seg_c[p])
        q = int(np.searchsorted(seg_c, b + SEGW, side="left"))
        cut = min(p + CHUNK, q, n)
        assert cut > p
        starts.append(p)
        ends.append(cut)
        bases.append(b)
        p = cut
    return (np.asarray(starts, np.int64), np.asarray(ends, np.int64),
            np.asarray(bases, np.int64))


def _host_prep(in_features, out_features, W1, b1, W2, b2,
               neighbors_index, neighbors_row_splits):
    rs = np.asarray(neighbors_row_splits).astype(np.int64)
    idx_all = np.asarray(neighbors_index).astype(np.int64)
    counts = np.diff(rs)
    seg_all = np.repeat(np.arange(M, dtype=np.int64), counts)

    bounds = [round(k * E / NCORES) for k in range(NCORES + 1)]

    cores = []
    nwin = 0
    for k in range(NCORES):
        lo, hi = bounds[k], bounds[k + 1]
        seg_c = seg_all[lo:hi]
        starts, ends, bases = _cut_chunks(seg_c)
        nch = starts.shape[0]
        nwin = max(nwin, -(-nch // CPW))
        cores.append((idx_all[lo:hi], seg_c, starts, ends, bases))

    inF = np.asarray(in_features, np.float32).astype(BF16)
    outF = np.asarray(out_features, np.float32).astype(BF16)

    w1 = np.asarray(W1, np.float32)
    w1cat = np.concatenate([w1, np.asarray(b1, np.float32).reshape(1, H)], 0)
    consts = dict(
        w1cat=np.ascontiguousarray(w1cat).astype(BF16),
        w2=np.asarray(W2, np.float32).astype(BF16),
    )

    nch_max = max(c[2].shape[0] for c in cores)

    in_maps = []
    metas = []
    for k in range(NCORES):
        idx_c, seg_c, starts, ends, bases = cores[k]
        n = idx_c.shape[0]
        nch = starts.shape[0]
        ncs = ends - starts
        chunk_ids = np.repeat(np.arange(nch, dtype=np.int64), ncs)
        within = np.arange(n, dtype=np.int64) - np.repeat(starts, ncs)
        slots = chunk_ids * CHUNK + within

        comb = np.zeros((KC, nwin * WIN), BF16)
        comb[0:C, slots] = inF[idx_c].T
        comb[C:2 * C, slots] = outF[seg_c].T
        comb[2 * C, slots] = np.float32(1.0)

        sloc = seg_c - bases[chunk_ids]
        assert int(sloc.max()) < SEGW
        sme = np.zeros((CHUNK, nwin * CPW * SEGW), FP8)
        sme[within, chunk_ids * SEGW + sloc] = np.float32(1.0)

        bases_full = np.zeros(nwin * CPW, np.int64)
        bases_full[:nch] = bases

        in_maps.append(dict(comb=comb, sme=sme, **consts))
        metas.append(dict(bases=bases_full, nch=nch))

    # real chunks in the final window (the rest are padding with all-zero
    # one-hots; their gelu can be skipped on-device)
    cpw_last = nch_max - (nwin - 1) * CPW if nwin >= 4 else CPW
    return in_maps, metas, nwin, cpw_last, counts


# ------------------------------------------------------------ device program

def _build_program(nwin, cpw_last=CPW):
    import concourse.bacc as bacc
    import concourse.mybir as mybir
    import concourse.tile as tile

    dt = mybir.dt
    nc = bacc.Bacc("TRN2", target_bir_lowering=False, debug=False)

    d_comb = nc.dram_tensor("comb", [KC, nwin * WIN], dt.bfloat16,
                            kind="ExternalInput")
    d_sme = nc.dram_tensor("sme", [CHUNK, nwin * CPW * SEGW], dt.float8e4,
                           kind="ExternalInput")
    d_w1cat = nc.dram_tensor("w1cat", [KC, H], dt.bfloat16,
                             kind="ExternalInput")
    d_w2 = nc.dram_tensor("w2", [H, O], dt.bfloat16, kind="ExternalInput")
    d_y = nc.dram_tensor("yout", [M2R, nwin * 2 * O], dt.float32,
                         kind="ExternalOutput")

    from contextlib import ExitStack

    with tile.TileContext(nc) as tc, ExitStack() as ctx:
        cpool = ctx.enter_context(tc.tile_pool(name="consts", bufs=1))
        gpool = ctx.enter_context(tc.tile_pool(name="stream", bufs=3))
        hpool = ctx.enter_context(tc.tile_pool(name="hsb", bufs=3))
        spool = ctx.enter_context(tc.tile_pool(name="small", bufs=3))
        ypool = ctx.enter_context(tc.tile_pool(name="ystage", bufs=3))
        hpsum = ctx.enter_context(tc.tile_pool(name="hpsum", bufs=2,
                                               space="PSUM"))
        wpsum = ctx.enter_context(tc.tile_pool(name="wpsum", bufs=2,
                                               space="PSUM"))

        w1_sb = cpool.tile([KC, H], dt.bfloat16, tag="w1")
        w2_sb = cpool.tile([H, O], dt.bfloat16, tag="w2")
        # consts issue on Act's (idle) sequencer so the first comb stream is
        # not queued behind them on SP.
        nc.scalar.dma_start(out=w1_sb[:], in_=d_w1cat[:])
        nc.scalar.dma_start(out=w2_sb[:], in_=d_w2[:])

        # Warm the Gelu activation table while the first streams are in
        # flight (table load is ~1.3us and otherwise serializes before the
        # first real gelu).
        warm = cpool.tile([1, 2], dt.bfloat16, tag="warm")
        nc.gpsimd.memset(warm[:], 0.0)
        nc.scalar.activation(warm[:], warm[:],
                             func=mybir.ActivationFunctionType.Gelu,
                             bias=0.0, scale=1.0)

        # group g covers windows [g*GRP, g*GRP+gsz); the last group may be
        # smaller than GRP when nwin is not a multiple of GRP.
        ngrp = -(-nwin // GRP)
        gsize = [min(GRP, nwin - g * GRP) for g in range(ngrp)]

        tiles = {}   # group -> (comb_t, sme_t)
        ysbs = {}    # group -> y_sb staging tile
        pend = None  # (k, h_ps, h_sb) waiting for its segsum/M2 phase

        def fetch_group(g):
            gsz = gsize[g]
            w0 = g * GRP
            comb_t = gpool.tile([KC, gsz * WIN], dt.bfloat16,
                                tag=f"comb{gsz}", name=f"comb{g}")
            if g == 0:
                # split per window so the first M1 starts after 1/GRP of the
                # stream has landed (range-tracked dependencies).
                for w in range(gsz):
                    nc.sync.dma_start(
                        out=comb_t[:, w * WIN:(w + 1) * WIN],
                        in_=d_comb[:, (w0 + w) * WIN:(w0 + w + 1) * WIN])
            else:
                nc.sync.dma_start(
                    out=comb_t[:],
                    in_=d_comb[:, w0 * WIN:(w0 + gsz) * WIN])
            sme_t = gpool.tile([CHUNK, gsz * CPW * SEGW], dt.float8e4,
                               tag=f"sme{gsz}", name=f"sme{g}")
            nc.gpsimd.dma_start(
                out=sme_t[:],
                in_=d_sme[:, w0 * CPW * SEGW:(w0 + gsz) * CPW * SEGW])
            tiles[g] = (comb_t, sme_t)

        def finish_window(k, h_ps, h_sb):
            g = k // GRP
            w = k % GRP
            gsz = gsize[g]
            sme_t = tiles[g][1]
            # the final window only carries cpw_last real chunks; the host
            # masks out all padding-chunk rows, so the device can skip them.
            nreal = cpw_last if k == nwin - 1 else CPW
            # combined scratch: hsT [128, CPW*16] f32 + y [96, 2, 64] f32 in
            # one PSUM bank.
            scr = wpsum.tile([CHUNK, CPW * SEGW + 2 * O], dt.float32,
                             tag="scr", name=f"scr{k}")
            hsT_ps = scr[:, 0:CPW * SEGW]
            for c in range(nreal):
                nc.tensor.matmul(
                    hsT_ps[:, c * SEGW:(c + 1) * SEGW],
                    lhsT=h_sb[:, c, :],
                    rhs=sme_t[:, (w * CPW + c) * SEGW:
                              (w * CPW + c + 1) * SEGW],
                    start=True, stop=True,
                    skip_group_check=True,
                )
            hsT_sb = spool.tile([H, nreal, SEGW], dt.bfloat16,
                                tag=f"hsTsb{nreal}", name=f"hsTsb{k}")
            nc.vector.tensor_copy(out=hsT_sb[:],
                                  in_=hsT_ps[:, 0:nreal * SEGW].rearrange(
                                      "p (a b) -> p a b", a=nreal))
            y_ps = scr[0:M2R, CPW * SEGW:].rearrange("p (a b) -> p a b", a=2)
            if g not in ysbs:
                ysbs[g] = ypool.tile([M2R, gsz * 2, O], dt.float32,
                                     tag=f"ysb{gsz}", name=f"ysb{g}")
            hw = CPW // 2
            halves = [(0, min(hw, nreal))]
            if nreal > hw:
                halves.append((hw, nreal))
            for s2, (a, b) in enumerate(halves):
                rows = (b - a) * SEGW
                nc.tensor.matmul(
                    y_ps[0:rows, s2, :],
                    lhsT=hsT_sb[:, a:b, :],
                    rhs=w2_sb[:],
                    start=True, stop=True,
                    skip_group_check=True,
                )
            if nreal == CPW:
                nc.vector.tensor_copy(out=ysbs[g][:, w * 2:(w + 1) * 2, :],
                                      in_=y_ps)
            else:
                for s2, (a, b) in enumerate(halves):
                    rows = (b - a) * SEGW
                    nc.vector.tensor_copy(
                        out=ysbs[g][0:rows, w * 2 + s2, :],
                        in_=y_ps[0:rows, s2, :])
            if w == gsz - 1:
                y0 = g * GRP * 2 * O
                nc.sync.dma_start(
                    out=d_y[:, y0:y0 + gsz * 2 * O],
                    in_=ysbs.pop(g)[:])
                tiles.pop(g)

        for k in range(nwin):
            g, w = k // GRP, k % GRP
            if w == 0:
                fetch_group(g)
            comb_t = tiles[g][0]
            h_ps = hpsum.tile([CHUNK, CPW, H], dt.float32, tag="h")
            for c in range(CPW):
                e0 = w * WIN + c * CHUNK
                nc.tensor.matmul(
                    h_ps[:, c, :],
                    lhsT=comb_t[:, e0:e0 + CHUNK],
                    rhs=w1_sb[:],
                    start=True, stop=True,
                )
            h_sb = hpool.tile([CHUNK, CPW, H], dt.bfloat16, tag="hsb")
            # the final window's padding chunks have all-zero one-hots, so
            # their (stale) h_sb contents never reach the output — gelu only
            # the real chunks.
            ngelu = cpw_last if k == nwin - 1 else CPW
            nc.scalar.activation(
                h_sb[:, 0:ngelu, :], h_ps[:, 0:ngelu, :],
                func=mybir.ActivationFunctionType.Gelu,
                bias=0.0, scale=1.0,
            )
            if pend is not None:
                finish_window(*pend)
            pend = (k, h_ps, h_sb)
        finish_window(*pend)

    nc.compile()
    return nc


# ------------------------------------------------------------------- runner

LAST_RESULT = None


def kernel(in_features, out_features, W1, b1, W2, b2,
           neighbors_index, neighbors_row_splits):
    import os
    from concourse.bass_utils import run_bass_kernel_spmd

    in_maps, metas, nwin, cpw_last, counts = _host_prep(
        in_features, out_features, W1, b1, W2, b2,
        neighbors_index, neighbors_row_splits,
    )

    key = (nwin, cpw_last)
    if key not in _prog_cache:
        _prog_cache[key] = _build_program(nwin, cpw_last)
    nc = _prog_cache[key]

    trace = bool(os.environ.get("KERNEL_TRACE"))
    if trace:
        try:
            import antenv.axon_hooks  # noqa: F401
        except ImportError:
            trace = False
    res = run_bass_kernel_spmd(nc, in_maps, core_ids=list(range(NCORES)),
                               trace=trace)
    global LAST_RESULT
    LAST_RESULT = res
    outs = res.results

    acc = np.zeros((M, O), np.float64)
    for k in range(NCORES):
        # yout [96, nwin*2, O] -> rows r = (w, s2, p) map to
        # chunk = w*CPW + s2*6 + p//16, seg = bases[chunk] + p % 16.
        y = np.asarray(outs[k]["yout"], np.float32)
        y = y.reshape(M2R, nwin, 2, O).transpose(1, 2, 0, 3)
        y = np.ascontiguousarray(y).reshape(nwin * 2 * M2R, O)
        p = np.tile(np.arange(M2R), nwin * 2)
        s2 = np.tile(np.repeat(np.arange(2), M2R), nwin)
        w = np.repeat(np.arange(nwin), 2 * M2R)
        chunk = w * CPW + s2 * (CPW // 2) + p // SEGW
        # padding-chunk rows (incl. the trimmed final window's stale staging)
        # are routed to a dummy bucket M that the [:M] truncation drops.
        gidx = np.where(chunk < metas[k]["nch"],
                        metas[k]["bases"][chunk] + p % SEGW, M)
        for o in range(O):
            # rows past a chunk's actual span are all-zero; indices may run
            # past M-1 for chunks near the end — truncate.
            acc[:, o] += np.bincount(gidx, weights=y[:, o].astype(np.float64),
                                     minlength=M)[:M]

    denom = np.maximum(counts, 1).astype(np.float64)
    out = (acc / denom[:, None]).astype(np.float32)
    b2v = np.asarray(b2, np.float32)
    out += b2v[None, :] * (counts > 0)[:, None].astype(np.float32)
    return out


# revision 34
# speedup vs baseline: 1.0062x; 1.0024x over previous
"""NeighborMLPConvLayer Trainium2 kernel (v3).

Strategy (8 NeuronCores, SPMD, edge-parallel):
  - Edges (sorted by destination segment) are split into 8 equal contiguous
    ranges; boundary segments are fixed up by a host-side overlap-add.
  - Per core, edges are packed into 128-edge "chunks"; a chunk never spans
    more than 16 distinct segments (cut + pad otherwise, which is rare at
    avg degree 32).  12 chunks form a 1536-slot window.
  - The HOST gathers features per edge into a dense bf16 stream
    comb[65, slots]: rows 0-31 in_features[idx], rows 32-63
    out_features[seg], row 64 ones (injects b1 via W1cat row 64).  Dense
    streams run at full DMA bandwidth (no per-row gather descriptors).
  - Device per chunk: h = comb_chunk.T @ W1cat (K=65) -> PSUM [128e, H];
    one gelu per window on Act (the bottleneck engine) -> SBUF bf16.
  - Segment-sum BEFORE W2 (linearity): hsT[H, 16] = h_chunk.T @ onehot16
    where onehot16 is a per-chunk [128e, 16s] fp8 one-hot (16 B/edge).
    Per window the 12 chunk slabs land in one PSUM scratch tile.
  - M2: y[96, O] = hsT_sb.T @ W2 twice per window (192 chunk-seg rows).
  - The loop is software-pipelined with a 1-window skew (PE is in-order:
    segsum(k-1), which waits on gelu(k-1), is emitted after M1(k) so the
    gelu hides behind the next window's M1).
  - Host: overlap-add chunk slabs (base + 16 rows each) into out[M, O]
    via per-column bincount, then divide by counts and add b2.
"""

import sys

sys.path.insert(0, "/opt/trn_rl_repo")

import numpy as np
import ml_dtypes

BF16 = ml_dtypes.bfloat16
FP8 = ml_dtypes.float8_e4m3

# Problem geometry (hardcoded per the task contract).
N = 50000
M = 50000
C = 32
H = 128
O = 64
E = 1_600_000
NCORES = 8

CHUNK = 128            # edges per chunk (PE partition dim)
SEGW = 16              # max segments spanned by one chunk (one-hot width)
CPW = 12               # chunks per window (PSUM: 12*128*4B = 3 banks)
WIN = CHUNK * CPW      # 1536 edge slots per window
GRP = 2                # windows per DMA group
KC = C + C + 1         # comb rows: rep(32) + slf(32) + ones(1)
M2R = CPW * SEGW // 2  # 96 output rows per M2 matmul

_prog_cache = {}


# ----------------------------------------------------------------- host prep

def _cut_chunks(seg_c):
    """Greedy 128-edge chunks, each spanning < SEGW segments.

    Returns (starts, ends, bases) arrays.
    """
    n = seg_c.shape[0]
    starts, ends, bases = [], [], []
    p = 0
    while p < n:
        b = int(seg_c[p])
        q = int(np.searchsorted(seg_c, b + SEGW, side="left"))
        cut = min(p + CHUNK, q, n)
        assert cut > p
        starts.append(p)
        ends.append(cut)
        bases.append(b)
        p = cut
    return (np.asarray(starts, np.int64), np.asarray(ends, np.int64),
            np.asarray(bases, np.int64))


def _host_prep(in_features, out_features, W1, b1, W2, b2,
               neighbors_index, neighbors_row_splits):
    rs = np.asarray(neighbors_row_splits).astype(np.int64)
    idx_all = np.asarray(neighbors_index).astype(np.int64)
    counts = np.diff(rs)
    seg_all = np.repeat(np.arange(M, dtype=np.int64), counts)

    bounds = [round(k * E / NCORES) for k in range(NCORES + 1)]

    cores = []
    nwin = 0
    for k in range(NCORES):
        lo, hi = bounds[k], bounds[k + 1]
        seg_c = seg_all[lo:hi]
        starts, ends, bases = _cut_chunks(seg_c)
        nch = starts.shape[0]
        nwin = max(nwin, -(-nch // CPW))
        cores.append((idx_all[lo:hi], seg_c, starts, ends, bases))

    inF = np.asarray(in_features, np.float32).astype(BF16)
    outF = np.asarray(out_features, np.float32).astype(BF16)

    w1 = np.asarray(W1, np.float32)
    w1cat = np.concatenate([w1, np.asarray(b1, np.float32).reshape(1, H)], 0)
    consts = dict(
        w1cat=np.ascontiguousarray(w1cat).astype(BF16),
        w2=np.asarray(W2, np.float32).astype(BF16),
    )

    nch_max = max(c[2].shape[0] for c in cores)

    in_maps = []
    metas = []
    for k in range(NCORES):
        idx_c, seg_c, starts, ends, bases = cores[k]
        n = idx_c.shape[0]
        nch = starts.shape[0]
        ncs = ends - starts
        chunk_ids = np.repeat(np.arange(nch, dtype=np.int64), ncs)
        within = np.arange(n, dtype=np.int64) - np.repeat(starts, ncs)
        slots = chunk_ids * CHUNK + within

        comb = np.zeros((KC, nwin * WIN), BF16)
        comb[0:C, slots] = inF[idx_c].T
        comb[C:2 * C, slots] = outF[seg_c].T
        comb[2 * C, slots] = np.float32(1.0)

        sloc = seg_c - bases[chunk_ids]
        assert int(sloc.max()) < SEGW
        sme = np.zeros((CHUNK, nwin * CPW * SEGW), FP8)
        sme[within, chunk_ids * SEGW + sloc] = np.float32(1.0)

        bases_full = np.zeros(nwin * CPW, np.int64)
        bases_full[:nch] = bases

        in_maps.append(dict(comb=comb, sme=sme, **consts))
        metas.append(dict(bases=bases_full, nch=nch))

    # real chunks in the final window (the rest are padding with all-zero
    # one-hots; their gelu can be skipped on-device)
    cpw_last = nch_max - (nwin - 1) * CPW if nwin >= 4 else CPW
    return in_maps, metas, nwin, cpw_last, counts


# ------------------------------------------------------------ device program

def _build_program(nwin, cpw_last=CPW):
    import concourse.bacc as bacc
    import concourse.mybir as mybir
    import concourse.tile as tile

    dt = mybir.dt
    nc = bacc.Bacc("TRN2", target_bir_lowering=False, debug=False)

    d_comb = nc.dram_tensor("comb", [KC, nwin * WIN], dt.bfloat16,
                            kind="ExternalInput")
    d_sme = nc.dram_tensor("sme", [CHUNK, nwin * CPW * SEGW], dt.float8e4,
                           kind="ExternalInput")
    d_w1cat = nc.dram_tensor("w1cat", [KC, H], dt.bfloat16,
                             kind="ExternalInput")
    d_w2 = nc.dram_tensor("w2", [H, O], dt.bfloat16, kind="ExternalInput")
    d_y = nc.dram_tensor("yout", [M2R, nwin * 2 * O], dt.float32,
                         kind="ExternalOutput")
    # the final window ships its segment sums pre-W2 (the host applies W2 to
    # these few rows), shortening the end-of-program serial chain.
    d_hlast = nc.dram_tensor("hlast", [H, cpw_last * SEGW], dt.bfloat16,
                             kind="ExternalOutput")

    from contextlib import ExitStack

    with tile.TileContext(nc) as tc, ExitStack() as ctx:
        cpool = ctx.enter_context(tc.tile_pool(name="consts", bufs=1))
        gpool = ctx.enter_context(tc.tile_pool(name="stream", bufs=3))
        hpool = ctx.enter_context(tc.tile_pool(name="hsb", bufs=3))
        spool = ctx.enter_context(tc.tile_pool(name="small", bufs=3))
        ypool = ctx.enter_context(tc.tile_pool(name="ystage", bufs=3))
        hpsum = ctx.enter_context(tc.tile_pool(name="hpsum", bufs=2,
                                               space="PSUM"))
        wpsum = ctx.enter_context(tc.tile_pool(name="wpsum", bufs=2,
                                               space="PSUM"))

        w1_sb = cpool.tile([KC, H], dt.bfloat16, tag="w1")
        w2_sb = cpool.tile([H, O], dt.bfloat16, tag="w2")
        # w1cat goes through Pool's SWDGE so it does not contend for the
        # HWDGE slot with the first comb stream (both gate the first M1);
        # w2 is not needed until the first finish_window, Act's queue is fine.
        nc.gpsimd.dma_start(out=w1_sb[:], in_=d_w1cat[:])
        nc.scalar.dma_start(out=w2_sb[:], in_=d_w2[:])

        # Warm the Gelu activation table while the first streams are in
        # flight (table load is ~1.3us and otherwise serializes before the
        # first real gelu).
        warm = cpool.tile([1, 2], dt.bfloat16, tag="warm")
        nc.gpsimd.memset(warm[:], 0.0)
        nc.scalar.activation(warm[:], warm[:],
                             func=mybir.ActivationFunctionType.Gelu,
                             bias=0.0, scale=1.0)

        # group g covers windows [g*GRP, g*GRP+gsz); the last group may be
        # smaller than GRP when nwin is not a multiple of GRP.
        ngrp = -(-nwin // GRP)
        gsize = [min(GRP, nwin - g * GRP) for g in range(ngrp)]

        tiles = {}   # group -> (comb_t, sme_t)
        ysbs = {}    # group -> y_sb staging tile
        pend = None  # (k, h_ps, h_sb) waiting for its segsum/M2 phase

        def fetch_group(g):
            gsz = gsize[g]
            w0 = g * GRP
            comb_t = gpool.tile([KC, gsz * WIN], dt.bfloat16,
                                tag=f"comb{gsz}", name=f"comb{g}")
            if g == 0:
                # split per window so the first M1 starts after 1/GRP of the
                # stream has landed (range-tracked dependencies).
                for w in range(gsz):
                    nc.sync.dma_start(
                        out=comb_t[:, w * WIN:(w + 1) * WIN],
                        in_=d_comb[:, (w0 + w) * WIN:(w0 + w + 1) * WIN])
            else:
                nc.sync.dma_start(
                    out=comb_t[:],
                    in_=d_comb[:, w0 * WIN:(w0 + gsz) * WIN])
            sme_t = gpool.tile([CHUNK, gsz * CPW * SEGW], dt.float8e4,
                               tag=f"sme{gsz}", name=f"sme{g}")
            nc.gpsimd.dma_start(
                out=sme_t[:],
                in_=d_sme[:, w0 * CPW * SEGW:(w0 + gsz) * CPW * SEGW])
            tiles[g] = (comb_t, sme_t)

        def finish_window(k, h_ps, h_sb):
            g = k // GRP
            w = k % GRP
            gsz = gsize[g]
            sme_t = tiles[g][1]
            # the final window only carries cpw_last real chunks; the host
            # masks out all padding-chunk rows, so the device can skip them.
            nreal = cpw_last if k == nwin - 1 else CPW
            # combined scratch: hsT [128, CPW*16] f32 + y [96, 2, 64] f32 in
            # one PSUM bank.
            scr = wpsum.tile([CHUNK, CPW * SEGW + 2 * O], dt.float32,
                             tag="scr", name=f"scr{k}")
            hsT_ps = scr[:, 0:CPW * SEGW]
            for c in range(nreal):
                nc.tensor.matmul(
                    hsT_ps[:, c * SEGW:(c + 1) * SEGW],
                    lhsT=h_sb[:, c, :],
                    rhs=sme_t[:, (w * CPW + c) * SEGW:
                              (w * CPW + c + 1) * SEGW],
                    start=True, stop=True,
                    skip_group_check=True,
                )
            hsT_sb = spool.tile([H, nreal, SEGW], dt.bfloat16,
                                tag=f"hsTsb{nreal}", name=f"hsTsb{k}")
            nc.vector.tensor_copy(out=hsT_sb[:],
                                  in_=hsT_ps[:, 0:nreal * SEGW].rearrange(
                                      "p (a b) -> p a b", a=nreal))
            if k == nwin - 1:
                # final window: ship hsT pre-W2 (host multiplies); also flush
                # any earlier windows of this group still staged.
                nc.sync.dma_start(out=d_hlast[:],
                                  in_=hsT_sb[:].rearrange("p a b -> p (a b)"))
                if w > 0:
                    y0 = g * GRP * 2 * O
                    nc.sync.dma_start(out=d_y[:, y0:y0 + w * 2 * O],
                                      in_=ysbs.pop(g)[:, 0:w * 2, :])
                tiles.pop(g)
                return
            y_ps = scr[0:M2R, CPW * SEGW:].rearrange("p (a b) -> p a b", a=2)
            if g not in ysbs:
                ysbs[g] = ypool.tile([M2R, gsz * 2, O], dt.float32,
                                     tag=f"ysb{gsz}", name=f"ysb{g}")
            hw = CPW // 2
            halves = [(0, min(hw, nreal))]
            if nreal > hw:
                halves.append((hw, nreal))
            for s2, (a, b) in enumerate(halves):
                rows = (b - a) * SEGW
                nc.tensor.matmul(
                    y_ps[0:rows, s2, :],
                    lhsT=hsT_sb[:, a:b, :],
                    rhs=w2_sb[:],
                    start=True, stop=True,
                    skip_group_check=True,
                )
            if nreal == CPW:
                nc.vector.tensor_copy(out=ysbs[g][:, w * 2:(w + 1) * 2, :],
                                      in_=y_ps)
            else:
                for s2, (a, b) in enumerate(halves):
                    rows = (b - a) * SEGW
                    nc.vector.tensor_copy(
                        out=ysbs[g][0:rows, w * 2 + s2, :],
                        in_=y_ps[0:rows, s2, :])
            if w == gsz - 1:
                y0 = g * GRP * 2 * O
                nc.sync.dma_start(
                    out=d_y[:, y0:y0 + gsz * 2 * O],
                    in_=ysbs.pop(g)[:])
                tiles.pop(g)

        for k in range(nwin):
            g, w = k // GRP, k % GRP
            if w == 0:
                fetch_group(g)
            comb_t = tiles[g][0]
            h_ps = hpsum.tile([CHUNK, CPW, H], dt.float32, tag="h")
            for c in range(CPW):
                e0 = w * WIN + c * CHUNK
                nc.tensor.matmul(
                    h_ps[:, c, :],
                    lhsT=comb_t[:, e0:e0 + CHUNK],
                    rhs=w1_sb[:],
                    start=True, stop=True,
                )
            h_sb = hpool.tile([CHUNK, CPW, H], dt.bfloat16, tag="hsb")
            # the final window's padding chunks have all-zero one-hots, so
            # their (stale) h_sb contents never reach the output — gelu only
            # the real chunks.
            ngelu = cpw_last if k == nwin - 1 else CPW
            nc.scalar.activation(
                h_sb[:, 0:ngelu, :], h_ps[:, 0:ngelu, :],
                func=mybir.ActivationFunctionType.Gelu,
                bias=0.0, scale=1.0,
            )
            if pend is not None:
                finish_window(*pend)
            pend = (k, h_ps, h_sb)
        finish_window(*pend)

    nc.compile()
    return nc


# ------------------------------------------------------------------- runner

LAST_RESULT = None


def kernel(in_features, out_features, W1, b1, W2, b2,
           neighbors_index, neighbors_row_splits):
    import os
    from concourse.bass_utils import run_bass_kernel_spmd

    in_maps, metas, nwin, cpw_last, counts = _host_prep(
        in_features, out_features, W1, b1, W2, b2,
        neighbors_index, neighbors_row_splits,
    )

    key = (nwin, cpw_last)
    if key not in _prog_cache:
        _prog_cache[key] = _build_program(nwin, cpw_last)
    nc = _prog_cache[key]

    trace = bool(os.environ.get("KERNEL_TRACE"))
    if trace:
        try:
            import antenv.axon_hooks  # noqa: F401
        except ImportError:
            trace = False
    res = run_bass_kernel_spmd(nc, in_maps, core_ids=list(range(NCORES)),
                               trace=trace)
    global LAST_RESULT
    LAST_RESULT = res
    outs = res.results

    acc = np.zeros((M, O), np.float64)
    for k in range(NCORES):
        # yout [96, nwin*2, O] -> rows r = (w, s2, p) map to
        # chunk = w*CPW + s2*6 + p//16, seg = bases[chunk] + p % 16.
        y = np.asarray(outs[k]["yout"], np.float32)
        y = y.reshape(M2R, nwin, 2, O).transpose(1, 2, 0, 3)
        y = np.ascontiguousarray(y).reshape(nwin * 2 * M2R, O)
        p = np.tile(np.arange(M2R), nwin * 2)
        s2 = np.tile(np.repeat(np.arange(2), M2R), nwin)
        w = np.repeat(np.arange(nwin), 2 * M2R)
        chunk = w * CPW + s2 * (CPW // 2) + p // SEGW
        # padding-chunk rows and the final window (shipped pre-W2 via hlast)
        # are routed to a dummy bucket M that the [:M] truncation drops.
        valid = (chunk < metas[k]["nch"]) & (w < nwin - 1)
        gidx = np.where(valid, metas[k]["bases"][chunk] + p % SEGW, M)

        # final window: host-side W2 on the shipped segment sums
        hl = np.asarray(outs[k]["hlast"], np.float32)  # [H, cpw_last*16]
        y_last = hl.T @ np.asarray(W2, np.float32)     # [cpw_last*16, O]
        cl = (nwin - 1) * CPW + np.arange(cpw_last * SEGW) // SEGW
        gidx_l = np.where(cl < metas[k]["nch"],
                          metas[k]["bases"][cl] + np.arange(
                              cpw_last * SEGW) % SEGW, M)
        y = np.concatenate([y, y_last], axis=0)
        gidx = np.concatenate([gidx, gidx_l])
        for o in range(O):
            # rows past a chunk's actual span are all-zero; indices may run
            # past M-1 for chunks near the end — truncate.
            acc[:, o] += np.bincount(gidx, weights=y[:, o].astype(np.float64),
                                     minlength=M)[:M]

    denom = np.maximum(counts, 1).astype(np.float64)
    out = (acc / denom[:, None]).astype(np.float32)
    b2v = np.asarray(b2, np.float32)
    out += b2v[None, :] * (counts > 0)[:, None].astype(np.float32)
    return out


# revision 36
# speedup vs baseline: 1.0072x; 1.0009x over previous
"""NeighborMLPConvLayer Trainium2 kernel (v3).

Strategy (8 NeuronCores, SPMD, edge-parallel):
  - Edges (sorted by destination segment) are split into 8 equal contiguous
    ranges; boundary segments are fixed up by a host-side overlap-add.
  - Per core, edges are packed into 128-edge "chunks"; a chunk never spans
    more than 16 distinct segments (cut + pad otherwise, which is rare at
    avg degree 32).  12 chunks form a 1536-slot window.
  - The HOST gathers features per edge into a dense bf16 stream
    comb[65, slots]: rows 0-31 in_features[idx], rows 32-63
    out_features[seg], row 64 ones (injects b1 via W1cat row 64).  Dense
    streams run at full DMA bandwidth (no per-row gather descriptors).
  - Device per chunk: h = comb_chunk.T @ W1cat (K=65) -> PSUM [128e, H];
    one gelu per window on Act (the bottleneck engine) -> SBUF bf16.
  - Segment-sum BEFORE W2 (linearity): hsT[H, 16] = h_chunk.T @ onehot16
    where onehot16 is a per-chunk [128e, 16s] fp8 one-hot (16 B/edge).
    Per window the 12 chunk slabs land in one PSUM scratch tile.
  - M2: y[96, O] = hsT_sb.T @ W2 twice per window (192 chunk-seg rows).
  - The loop is software-pipelined with a 1-window skew (PE is in-order:
    segsum(k-1), which waits on gelu(k-1), is emitted after M1(k) so the
    gelu hides behind the next window's M1).
  - Host: overlap-add chunk slabs (base + 16 rows each) into out[M, O]
    via per-column bincount, then divide by counts and add b2.
"""

import sys

sys.path.insert(0, "/opt/trn_rl_repo")

import numpy as np
import ml_dtypes

BF16 = ml_dtypes.bfloat16
FP8 = ml_dtypes.float8_e4m3

# Problem geometry (hardcoded per the task contract).
N = 50000
M = 50000
C = 32
H = 128
O = 64
E = 1_600_000
NCORES = 8

CHUNK = 128            # edges per chunk (PE partition dim)
SEGW = 16              # max segments spanned by one chunk (one-hot width)
CPW = 12               # chunks per window (PSUM: 12*128*4B = 3 banks)
WIN = CHUNK * CPW      # 1536 edge slots per window
GRP = 2                # windows per DMA group
KC = C + C + 1         # comb rows: rep(32) + slf(32) + ones(1)
M2R = CPW * SEGW // 2  # 96 output rows per M2 matmul

_prog_cache = {}


# ----------------------------------------------------------------- host prep

def _cut_chunks(seg_c):
    """Greedy 128-edge chunks, each spanning < SEGW segments.

    Returns (starts, ends, bases) arrays.
    """
    n = seg_c.shape[0]
    starts, ends, bases = [], [], []
    p = 0
    while p < n:
        b = int(seg_c[p])
        q = int(np.searchsorted(seg_c, b + SEGW, side="left"))
        cut = min(p + CHUNK, q, n)
        assert cut > p
        starts.append(p)
        ends.append(cut)
        bases.append(b)
        p = cut
    return (np.asarray(starts, np.int64), np.asarray(ends, np.int64),
            np.asarray(bases, np.int64))


def _host_prep(in_features, out_features, W1, b1, W2, b2,
               neighbors_index, neighbors_row_splits):
    rs = np.asarray(neighbors_row_splits).astype(np.int64)
    idx_all = np.asarray(neighbors_index).astype(np.int64)
    counts = np.diff(rs)
    seg_all = np.repeat(np.arange(M, dtype=np.int64), counts)

    bounds = [round(k * E / NCORES) for k in range(NCORES + 1)]

    cores = []
    nwin = 0
    for k in range(NCORES):
        lo, hi = bounds[k], bounds[k + 1]
        seg_c = seg_all[lo:hi]
        starts, ends, bases = _cut_chunks(seg_c)
        nch = starts.shape[0]
        nwin = max(nwin, -(-nch // CPW))
        cores.append((idx_all[lo:hi], seg_c, starts, ends, bases))

    inF = np.asarray(in_features, np.float32).astype(BF16)
    outF = np.asarray(out_features, np.float32).astype(BF16)

    w1 = np.asarray(W1, np.float32)
    w1cat = np.concatenate([w1, np.asarray(b1, np.float32).reshape(1, H)], 0)
    consts = dict(
        w1cat=np.ascontiguousarray(w1cat).astype(BF16),
        w2=np.asarray(W2, np.float32).astype(BF16),
    )

    nch_max = max(c[2].shape[0] for c in cores)

    in_maps = []
    metas = []
    for k in range(NCORES):
        idx_c, seg_c, starts, ends, bases = cores[k]
        n = idx_c.shape[0]
        nch = starts.shape[0]
        ncs = ends - starts
        chunk_ids = np.repeat(np.arange(nch, dtype=np.int64), ncs)
        within = np.arange(n, dtype=np.int64) - np.repeat(starts, ncs)
        slots = chunk_ids * CHUNK + within

        comb = np.zeros((KC, nwin * WIN), BF16)
        comb[0:C, slots] = inF[idx_c].T
        comb[C:2 * C, slots] = outF[seg_c].T
        comb[2 * C, slots] = np.float32(1.0)

        sloc = seg_c - bases[chunk_ids]
        assert int(sloc.max()) < SEGW
        sme = np.zeros((CHUNK, nwin * CPW * SEGW), FP8)
        sme[within, chunk_ids * SEGW + sloc] = np.float32(1.0)

        bases_full = np.zeros(nwin * CPW, np.int64)
        bases_full[:nch] = bases

        in_maps.append(dict(comb=comb, sme=sme, **consts))
        metas.append(dict(bases=bases_full, nch=nch))

    # real chunks in the final window (the rest are padding with all-zero
    # one-hots; their gelu can be skipped on-device)
    cpw_last = nch_max - (nwin - 1) * CPW if nwin >= 4 else CPW
    return in_maps, metas, nwin, cpw_last, counts


# ------------------------------------------------------------ device program

def _build_program(nwin, cpw_last=CPW):
    import concourse.bacc as bacc
    import concourse.mybir as mybir
    import concourse.tile as tile

    dt = mybir.dt
    nc = bacc.Bacc("TRN2", target_bir_lowering=False, debug=False)

    d_comb = nc.dram_tensor("comb", [KC, nwin * WIN], dt.bfloat16,
                            kind="ExternalInput")
    d_sme = nc.dram_tensor("sme", [CHUNK, nwin * CPW * SEGW], dt.float8e4,
                           kind="ExternalInput")
    d_w1cat = nc.dram_tensor("w1cat", [KC, H], dt.bfloat16,
                             kind="ExternalInput")
    d_w2 = nc.dram_tensor("w2", [H, O], dt.bfloat16, kind="ExternalInput")
    d_y = nc.dram_tensor("yout", [M2R, nwin * 2 * O], dt.float32,
                         kind="ExternalOutput")
    # the final window ships its segment sums pre-W2 (the host applies W2 to
    # these few rows), shortening the end-of-program serial chain.
    d_hlast = nc.dram_tensor("hlast", [H, cpw_last * SEGW], dt.bfloat16,
                             kind="ExternalOutput")

    from contextlib import ExitStack

    with tile.TileContext(nc) as tc, ExitStack() as ctx:
        cpool = ctx.enter_context(tc.tile_pool(name="consts", bufs=1))
        gpool = ctx.enter_context(tc.tile_pool(name="stream", bufs=3))
        hpool = ctx.enter_context(tc.tile_pool(name="hsb", bufs=3))
        spool = ctx.enter_context(tc.tile_pool(name="small", bufs=3))
        ypool = ctx.enter_context(tc.tile_pool(name="ystage", bufs=3))
        hpsum = ctx.enter_context(tc.tile_pool(name="hpsum", bufs=2,
                                               space="PSUM"))
        wpsum = ctx.enter_context(tc.tile_pool(name="wpsum", bufs=2,
                                               space="PSUM"))

        w1_sb = cpool.tile([KC, H], dt.bfloat16, tag="w1")
        w2_sb = cpool.tile([H, O], dt.bfloat16, tag="w2")
        # w1cat goes through Pool's SWDGE so it does not contend for the
        # HWDGE slot with the first comb stream (both gate the first M1);
        # w2 is not needed until the first finish_window, Act's queue is fine.
        nc.gpsimd.dma_start(out=w1_sb[:], in_=d_w1cat[:])
        nc.scalar.dma_start(out=w2_sb[:], in_=d_w2[:])

        # Warm the Gelu activation table while the first streams are in
        # flight (table load is ~1.3us and otherwise serializes before the
        # first real gelu).
        warm = cpool.tile([1, 2], dt.bfloat16, tag="warm")
        nc.gpsimd.memset(warm[:], 0.0)
        nc.scalar.activation(warm[:], warm[:],
                             func=mybir.ActivationFunctionType.Gelu,
                             bias=0.0, scale=1.0)

        # group g covers windows [g*GRP, g*GRP+gsz); the last group may be
        # smaller than GRP when nwin is not a multiple of GRP.
        ngrp = -(-nwin // GRP)
        gsize = [min(GRP, nwin - g * GRP) for g in range(ngrp)]

        tiles = {}   # group -> (comb_t, sme_t)
        ysbs = {}    # group -> y_sb staging tile
        pend = None  # (k, h_ps, h_sb) waiting for its segsum/M2 phase

        def fetch_group(g):
            gsz = gsize[g]
            w0 = g * GRP
            comb_t = gpool.tile([KC, gsz * WIN], dt.bfloat16,
                                tag=f"comb{gsz}", name=f"comb{g}")
            if g == 0:
                # split per window so the first M1 starts after 1/GRP of the
                # stream has landed (range-tracked dependencies).
                for w in range(gsz):
                    nc.sync.dma_start(
                        out=comb_t[:, w * WIN:(w + 1) * WIN],
                        in_=d_comb[:, (w0 + w) * WIN:(w0 + w + 1) * WIN])
            else:
                nc.sync.dma_start(
                    out=comb_t[:],
                    in_=d_comb[:, w0 * WIN:(w0 + gsz) * WIN])
            sme_t = gpool.tile([CHUNK, gsz * CPW * SEGW], dt.float8e4,
                               tag=f"sme{gsz}", name=f"sme{g}")
            nc.gpsimd.dma_start(
                out=sme_t[:],
                in_=d_sme[:, w0 * CPW * SEGW:(w0 + gsz) * CPW * SEGW])
            tiles[g] = (comb_t, sme_t)

        def finish_window(k, h_ps, h_sb):
            g = k // GRP
            w = k % GRP
            gsz = gsize[g]
            sme_t = tiles[g][1]
            # the final window only carries cpw_last real chunks; the host
            # masks out all padding-chunk rows, so the device can skip them.
            nreal = cpw_last if k == nwin - 1 else CPW
            # combined scratch: hsT [128, CPW*16] f32 + y [96, 2, 64] f32 in
            # one PSUM bank.
            scr = wpsum.tile([CHUNK, CPW * SEGW + 2 * O], dt.float32,
                             tag="scr", name=f"scr{k}")
            hsT_ps = scr[:, 0:CPW * SEGW]
            for c in range(nreal):
                nc.tensor.matmul(
                    hsT_ps[:, c * SEGW:(c + 1) * SEGW],
                    lhsT=h_sb[:, c, :],
                    rhs=sme_t[:, (w * CPW + c) * SEGW:
                              (w * CPW + c + 1) * SEGW],
                    start=True, stop=True,
                    skip_group_check=True,
                )
            hsT_sb = spool.tile([H, nreal, SEGW], dt.bfloat16,
                                tag=f"hsTsb{nreal}", name=f"hsTsb{k}")
            nc.vector.tensor_copy(out=hsT_sb[:],
                                  in_=hsT_ps[:, 0:nreal * SEGW].rearrange(
                                      "p (a b) -> p a b", a=nreal))
            if k == nwin - 1:
                # final window: ship hsT pre-W2 (host multiplies).  SWDGE
                # (Pool) path keeps it off the HWDGE queue that the last
                # yout flush needs.
                nc.gpsimd.dma_start(
                    out=d_hlast[:],
                    in_=hsT_sb[:].rearrange("p a b -> p (a b)"))
                tiles.pop(g)
                return
            y_ps = scr[0:M2R, CPW * SEGW:].rearrange("p (a b) -> p a b", a=2)
            if g not in ysbs:
                ysbs[g] = ypool.tile([M2R, gsz * 2, O], dt.float32,
                                     tag=f"ysb{gsz}", name=f"ysb{g}")
            hw = CPW // 2
            halves = [(0, min(hw, nreal))]
            if nreal > hw:
                halves.append((hw, nreal))
            for s2, (a, b) in enumerate(halves):
                rows = (b - a) * SEGW
                nc.tensor.matmul(
                    y_ps[0:rows, s2, :],
                    lhsT=hsT_sb[:, a:b, :],
                    rhs=w2_sb[:],
                    start=True, stop=True,
                    skip_group_check=True,
                )
            if nreal == CPW:
                nc.vector.tensor_copy(out=ysbs[g][:, w * 2:(w + 1) * 2, :],
                                      in_=y_ps)
            else:
                for s2, (a, b) in enumerate(halves):
                    rows = (b - a) * SEGW
                    nc.vector.tensor_copy(
                        out=ysbs[g][0:rows, w * 2 + s2, :],
                        in_=y_ps[0:rows, s2, :])
            if g >= ngrp - 2:
                # the last two groups finish after the final gelu (pipeline
                # skew); flush per window so only the final window's small
                # slab sits on the end-of-program critical path.
                y0 = (g * GRP + w) * 2 * O
                nc.sync.dma_start(out=d_y[:, y0:y0 + 2 * O],
                                  in_=ysbs[g][:, w * 2:(w + 1) * 2, :])
                if w == gsz - 1:
                    ysbs.pop(g)
                    tiles.pop(g)
            elif w == gsz - 1:
                y0 = g * GRP * 2 * O
                nc.sync.dma_start(
                    out=d_y[:, y0:y0 + gsz * 2 * O],
                    in_=ysbs.pop(g)[:])
                tiles.pop(g)

        for k in range(nwin):
            g, w = k // GRP, k % GRP
            if w == 0:
                fetch_group(g)
            comb_t = tiles[g][0]
            h_ps = hpsum.tile([CHUNK, CPW, H], dt.float32, tag="h")
            for c in range(CPW):
                e0 = w * WIN + c * CHUNK
                nc.tensor.matmul(
                    h_ps[:, c, :],
                    lhsT=comb_t[:, e0:e0 + CHUNK],
                    rhs=w1_sb[:],
                    start=True, stop=True,
                )
            h_sb = hpool.tile([CHUNK, CPW, H], dt.bfloat16, tag="hsb")
            # the final window's padding chunks have all-zero one-hots, so
            # their (stale) h_sb contents never reach the output — gelu only
            # the real chunks.
            ngelu = cpw_last if k == nwin - 1 else CPW
            nc.scalar.activation(
                h_sb[:, 0:ngelu, :], h_ps[:, 0:ngelu, :],
                func=mybir.ActivationFunctionType.Gelu,
                bias=0.0, scale=1.0,
            )
            if pend is not None:
                finish_window(*pend)
            pend = (k, h_ps, h_sb)
        finish_window(*pend)

    nc.compile()
    return nc


# ------------------------------------------------------------------- runner

LAST_RESULT = None


def kernel(in_features, out_features, W1, b1, W2, b2,
           neighbors_index, neighbors_row_splits):
    import os
    from concourse.bass_utils import run_bass_kernel_spmd

    in_maps, metas, nwin, cpw_last, counts = _host_prep(
        in_features, out_features, W1, b1, W2, b2,
        neighbors_index, neighbors_row_splits,
    )

    key = (nwin, cpw_last)
    if key not in _prog_cache:
        _prog_cache[key] = _build_program(nwin, cpw_last)
    nc = _prog_cache[key]

    trace = bool(os.environ.get("KERNEL_TRACE"))
    if trace:
        try:
            import antenv.axon_hooks  # noqa: F401
        except ImportError:
            trace = False
    res = run_bass_kernel_spmd(nc, in_maps, core_ids=list(range(NCORES)),
                               trace=trace)
    global LAST_RESULT
    LAST_RESULT = res
    outs = res.results

    acc = np.zeros((M, O), np.float64)
    for k in range(NCORES):
        # yout [96, nwin*2, O] -> rows r = (w, s2, p) map to
        # chunk = w*CPW + s2*6 + p//16, seg = bases[chunk] + p % 16.
        y = np.asarray(outs[k]["yout"], np.float32)
        y = y.reshape(M2R, nwin, 2, O).transpose(1, 2, 0, 3)
        y = np.ascontiguousarray(y).reshape(nwin * 2 * M2R, O)
        p = np.tile(np.arange(M2R), nwin * 2)
        s2 = np.tile(np.repeat(np.arange(2), M2R), nwin)
        w = np.repeat(np.arange(nwin), 2 * M2R)
        chunk = w * CPW + s2 * (CPW // 2) + p // SEGW
        # padding-chunk rows and the final window (shipped pre-W2 via hlast)
        # are routed to a dummy bucket M that the [:M] truncation drops.
        valid = (chunk < metas[k]["nch"]) & (w < nwin - 1)
        gidx = np.where(valid, metas[k]["bases"][chunk] + p % SEGW, M)

        # final window: host-side W2 on the shipped segment sums
        hl = np.asarray(outs[k]["hlast"], np.float32)  # [H, cpw_last*16]
        y_last = hl.T @ np.asarray(W2, np.float32)     # [cpw_last*16, O]
        cl = (nwin - 1) * CPW + np.arange(cpw_last * SEGW) // SEGW
        gidx_l = np.where(cl < metas[k]["nch"],
                          metas[k]["bases"][cl] + np.arange(
                              cpw_last * SEGW) % SEGW, M)
        y = np.concatenate([y, y_last], axis=0)
        gidx = np.concatenate([gidx, gidx_l])
        for o in range(O):
            # rows past a chunk's actual span are all-zero; indices may run
            # past M-1 for chunks near the end — truncate.
            acc[:, o] += np.bincount(gidx, weights=y[:, o].astype(np.float64),
                                     minlength=M)[:M]

    denom = np.maximum(counts, 1).astype(np.float64)
    out = (acc / denom[:, None]).astype(np.float32)
    b2v = np.asarray(b2, np.float32)
    out += b2v[None, :] * (counts > 0)[:, None].astype(np.float32)
    return out
